# revision 20
# baseline (speedup 1.0000x reference)
"""Trainium2 Bass kernel for nn_Conv2d_22222160789797.

Conv2d: x [32,128,56,56] f32, weight [256,128,3,3] (OIHW), stride 1, pad 1
-> out [32,256,56,56] f32.

Strategy: data-parallel over batch across 8 cores (4 images/core). Per core,
the conv is 9 accumulating matmuls per output tile: contract over in-channels
(partition dim K=128) with the weight slice for each (kh,kw) tap as the
stationary operand and a shifted window of the zero-padded input as the moving
operand. Inputs are cast to bf16 on the host (~2e-3 max rel err, limit 2e-2).

Loop order is tap-outer / chunk-inner: each (image, oc-half) group keeps 7
PSUM banks live (one per 8-row output chunk, N=448 fp32 <= 512 bank limit)
and sweeps the 9 taps over all 7 chunks, so the stationary weight is reused
across 7 consecutive matmuls and LDWEIGHTS hides under the 448-cycle stream.
Measured steady-state matmul issue gap: 192ns (187ns streaming floor); with
per-matmul weight swaps it was 237ns, with fp32r 4-byte weights 253ns.

Head/tail plumbing (all measured bottlenecks in earlier traces):
- sync (HWDGE) ring carries input loads in first-needed order: weight half 0
  (host-relaid so each half is contiguous, 2304B/partition), image-0 as
  three contiguous row-slabs (each gates only its 2-3 chunks), weight half
  1, then images 1-3 whole (6728B/partition) into dedicated buffers so no
  image DMA ever queues behind output DMAs. SWDGE (gpsimd) and the ACT ring
  measured 2-5us slower to first byte for the critical head transfers.
- PSUM evacuation alternates DVE/ACT per chunk so bank-free keeps pace with
  the next group's matmuls; output DMAs split across the sync/ACT rings
  (even/odd chunks) to halve the end-of-kernel issue serialization. Outputs
  are written bf16 (host upcasts) to halve the output-drain time.
- dummy matmuls on a zeroed scratch tile bridge the initial DMA wait so the
  PE HAM clock gate is at full rate when the real matmuls start (a >3.4us
  PE-idle gap re-throttles the clock to 1.2GHz for ~4us).
"""

import ml_dtypes
import numpy as np

import concourse.tile as tile
from concourse import bacc, mybir
from concourse.bass_utils import run_bass_kernel_spmd

N_CORES = 8
B, IC, H, W = 32, 128, 56, 56
OC, KH, KW = 256, 3, 3
BPC = B // N_CORES          # images per core
PH, PW = H + 2, W + 2       # padded 58x58
ROWS_PER_CHUNK = 8
N_CHUNKS = H // ROWS_PER_CHUNK  # 7
OC_HALVES = OC // 128       # 2
NTAPS = KH * KW

_f32 = mybir.dt.float32
_bf16 = mybir.dt.bfloat16
_bf16_np = ml_dtypes.bfloat16

_compiled_nc = None

N_WARMUP = 9  # dummy matmuls covering the initial DMA wait (~3.3us cold)


def _build(warmup=N_WARMUP):
    nc = bacc.Bacc("TRN2", target_bir_lowering=False, debug=False)
    x_d = nc.dram_tensor("x", [BPC, IC, PH, PW], _bf16, kind="ExternalInput")
    w_d = nc.dram_tensor("w", [IC, OC_HALVES * NTAPS * 128], _bf16,
                         kind="ExternalInput")
    o_d = nc.dram_tensor("out", [BPC, OC, H, W], _bf16, kind="ExternalOutput")
    w4 = w_d[:].rearrange("p (h k c) -> p h k c", h=OC_HALVES, k=NTAPS, c=128)

    with tile.TileContext(nc) as tc:
        with (
            tc.tile_pool(name="w", bufs=1) as wpool,
            tc.tile_pool(name="x", bufs=1) as xpool,
            tc.tile_pool(name="o", bufs=1) as opool,
            tc.tile_pool(name="ps", bufs=8, space="PSUM") as pspool,
        ):
            if warmup:
                wscr = wpool.tile([128, 128], _bf16, name="wscr", tag="wscr")
                xscr = wpool.tile([128, ROWS_PER_CHUNK * W], _bf16,
                                  name="xscr", tag="xscr")
                nc.gpsimd.memset(wscr[:], 0.0)
                nc.gpsimd.memset(xscr[:], 0.0)
                pwarm = pspool.tile([128, ROWS_PER_CHUNK * W], _f32,
                                    name="pwarm", tag="ps")
                for _ in range(warmup):
                    nc.tensor.matmul(pwarm[:], wscr[:], xscr[:],
                                     start=True, stop=True)

            # sync (HWDGE) ring order = first-needed order: weight half 0,
            # then image-0 in three contiguous row-slabs (each gates only its
            # chunks), then weight half 1, then whole images 1-3
            wh = []
            for half in range(OC_HALVES):
                t = wpool.tile([IC, NTAPS, 128], _bf16, name=f"wh{half}",
                               tag=f"wh{half}")
                wh.append(t)
            nc.sync.dma_start(wh[0][:], w4[:, 0])

            def tap(half, k):
                return wh[half][:, k, :]

            # slabs: rows 0-17 (chunks 0-1), 16-41 (chunks 2-4),
            # 40-57 (chunks 5-6)
            slab_rows = [(0, 18), (16, 26), (40, 18)]
            slabs = []
            for si, (r0, nr) in enumerate(slab_rows):
                s = xpool.tile([IC, nr, PW], _bf16, name=f"x0s{si}",
                               tag=f"x0s{si}")
                nc.sync.dma_start(s[:], x_d[0, :, r0 : r0 + nr, :])
                slabs.append(s)
            nc.sync.dma_start(wh[1][:], w4[:, 1])

            def img0_rhs(ch, kh, kw):
                r = ch * ROWS_PER_CHUNK + kh
                si = 0 if ch < 2 else (1 if ch < 5 else 2)
                r -= slab_rows[si][0]
                return slabs[si][:, r : r + ROWS_PER_CHUNK, kw : kw + W]

            # images 1-3 fully prefetched up front (own buffers, no reuse
            # waits) so no image DMA ever queues behind output DMAs on the
            # sync ring
            rhs_fns = [img0_rhs]
            for img in range(1, BPC):
                xt = xpool.tile([IC, PH, PW], _bf16, name=f"xt{img}",
                                tag=f"xt{img}")
                nc.sync.dma_start(xt[:], x_d[img])

                def rhs_of(ch, kh, kw, _xt=xt):
                    r = ch * ROWS_PER_CHUNK + kh
                    return _xt[:, r : r + ROWS_PER_CHUNK, kw : kw + W]

                rhs_fns.append(rhs_of)

            for img in range(BPC):
                rhs_of = rhs_fns[img]
                for half in range(OC_HALVES):
                    pss = []
                    for ch in range(N_CHUNKS):
                        ps = pspool.tile([128, ROWS_PER_CHUNK, W], _f32,
                                         name="ps", tag="ps")
                        pss.append(ps)
                    for k in range(NTAPS):
                        kh, kw = divmod(k, KW)
                        for ch in range(N_CHUNKS):
                            # padded rows 0 and 57 are all-zero: trim them
                            # off the border chunks' windows (N 448 -> 392).
                            # The skipped output rows are first-written by a
                            # later tap (has_written bit clear -> overwrite).
                            lo, nr = 0, ROWS_PER_CHUNK
                            if ch == 0 and kh == 0:
                                lo, nr = 1, ROWS_PER_CHUNK - 1
                            elif ch == N_CHUNKS - 1 and kh == KH - 1:
                                nr = ROWS_PER_CHUNK - 1
                            rhs = rhs_of(ch, kh, kw)
                            nc.tensor.matmul(
                                pss[ch][:, lo : lo + nr, :],
                                tap(half, k),
                                rhs[:, lo : lo + nr, :],
                                start=(k == 0),
                                stop=(k == NTAPS - 1),
                            )
                    for ch in range(N_CHUNKS):
                        r0 = ch * ROWS_PER_CHUNK
                        if ch % 2 == 0:
                            ot = opool.tile([128, ROWS_PER_CHUNK, W], _bf16,
                                            name="otv", tag="otv", bufs=4)
                            nc.vector.tensor_copy(ot[:], pss[ch][:])
                            ring = nc.sync
                        else:
                            ot = opool.tile([128, ROWS_PER_CHUNK, W], _bf16,
                                            name="ots", tag="ots", bufs=4)
                            nc.scalar.copy(ot[:], pss[ch][:])
                            ring = nc.scalar
                        ring.dma_start(
                            o_d[img, half * 128 : half * 128 + 128,
                                r0 : r0 + ROWS_PER_CHUNK, :],
                            ot[:],
                        )
    nc.compile()
    return nc


def _get_nc():
    global _compiled_nc
    if _compiled_nc is None:
        _compiled_nc = _build()
    return _compiled_nc


def _prep_inputs(x, weight):
    x = np.asarray(x, dtype=np.float32)
    weight = np.asarray(weight, dtype=np.float32)
    xp = np.zeros((B, IC, PH, PW), dtype=_bf16_np)
    xp[:, :, 1 : H + 1, 1 : W + 1] = x
    # [oc, ic, kh, kw] -> [ic, oc-half, kh*kw, 128] -> [ic, flat]
    wt = weight.transpose(1, 0, 2, 3).reshape(IC, OC_HALVES, 128, NTAPS)
    wt = np.ascontiguousarray(wt.transpose(0, 1, 3, 2).astype(_bf16_np))
    wt = wt.reshape(IC, OC_HALVES * NTAPS * 128)
    in_maps = [
        {"x": np.ascontiguousarray(xp[c * BPC : (c + 1) * BPC]), "w": wt}
        for c in range(N_CORES)
    ]
    return in_maps


def _run(x, weight, trace=False):
    nc = _get_nc()
    in_maps = _prep_inputs(x, weight)
    res = run_bass_kernel_spmd(nc, in_maps, list(range(N_CORES)), trace=trace)
    out = np.concatenate(
        [np.asarray(res.results[c]["out"]) for c in range(N_CORES)], axis=0
    ).astype(np.float32)
    return out, res


def kernel(x, weight):
    out, _ = _run(x, weight)
    return out
